# revision 11
# baseline (speedup 1.0000x reference)
"""BipartPool GATv2 pooling kernel for trn2, 8-core SPMD, graph-sharded.

Decomposition (per core, 8 graphs = 4096 nodes, 128 centroids):
  A[n,c,h] = sum_C att[h,C] * leaky(z),  z = Xl[n,h,C] + xr[c,h,C] + b_l[h,C]
           = 0.4 * sum_C att*|z|  + 0.6*(P[n,h] + Q[c,h]),   P = sum_C att*Xl
  V_c = |Xl + (xr_c+b_l)|  computed on ACT (Abs) / DVE (tensor_scalar add+abs_max),
  PE reduces V_c with block-diagonal att weights, accumulating all (c,h) rows plus
  the 1.5*P term into one PSUM tile [64,512] per graph; ACT Exp evacuates with
  scale 0.4 and per-partition bias 0.6*Q, accum_out gives the softmax denominator.
  Self-loop edges (PyG add_self_loops crosses graphs) enter as host-computed
  exp(A_self) terms; centroid row d<16 keeps no self term (it duplicates the
  masked dense edge).  Aggregation w^T @ Xl runs per (graph, head) on PE after a
  PE transpose of the normalized weights; head-mean + bias folds into the final
  ACT evacuation (scale 0.25, per-partition bias_eff).
"""

import numpy as np

NUM_GRAPHS = 64
PTS = 512
RATIO = 16
HEADS = 4
C = 64
NEG = 0.2
NCORES = 8
GPC = NUM_GRAPHS // NCORES          # graphs per core = 8
NPC = GPC * PTS                     # nodes per core = 4096
DPC = GPC * RATIO                   # centroids per core = 128

_CACHE = {}


def _build_program():
    import concourse.bass as bass
    import concourse.bacc as bacc
    import concourse.mybir as mybir
    from concourse.tile import TileContext

    f32 = mybir.dt.float32
    bf16 = mybir.dt.bfloat16

    nc = bacc.Bacc(None, target_bir_lowering=True)
    # per-core tensors
    xT = nc.declare_dram_parameter("xT", [C, NPC], bf16, isOutput=False)
    eselfT = nc.declare_dram_parameter("eselfT", [64, GPC], f32, isOutput=False)
    xlselfR = nc.declare_dram_parameter("xlselfR", [RATIO, GPC * HEADS * C], bf16, isOutput=False)
    # shared tensors
    Wl = nc.declare_dram_parameter("Wl", [C, HEADS * C], bf16, isOutput=False)
    scoreb = nc.declare_dram_parameter("scoreb", [128, 2 * RATIO], f32, isOutput=False)
    attBD = nc.declare_dram_parameter("attBD", [128, 32 * 64], bf16, isOutput=False)
    attP = nc.declare_dram_parameter("attP", [128, 2 * 64], bf16, isOutput=False)
    qb = nc.declare_dram_parameter("qb", [64, 1], f32, isOutput=False)
    ipat = nc.declare_dram_parameter("ipat", [64, RATIO], f32, isOutput=False)
    beff = nc.declare_dram_parameter("beff", [C, 1], f32, isOutput=False)
    ident = nc.declare_dram_parameter("ident", [64, 64], bf16, isOutput=False)
    outT = nc.declare_dram_parameter("outT", [C, DPC], f32, isOutput=True)

    NCHUNK = NPC // PTS  # 8 chunks of 512 = one graph each

    with TileContext(nc) as tc:
        with (
            tc.tile_pool(name="const", bufs=1) as cpool,
            tc.tile_pool(name="big", bufs=1) as bpool,
            tc.tile_pool(name="v", bufs=3) as vpool,
            tc.tile_pool(name="w", bufs=2) as wpool,
            tc.tile_pool(name="pa", bufs=2, space="PSUM") as pa_pool,
            tc.tile_pool(name="pt", bufs=2, space="PSUM") as pt_pool,
            tc.tile_pool(name="pg", bufs=2, space="PSUM") as pg_pool,
        ):
            # ---- load constants / inputs
            xT_sb = cpool.tile([C, NPC], bf16)
            for k in range(NPC // PTS):
                nc.sync.dma_start(xT_sb[:, k * PTS:(k + 1) * PTS],
                                  xT[:, k * PTS:(k + 1) * PTS])
            Wl_sb = cpool.tile([C, HEADS * C], bf16)
            nc.sync.dma_start(Wl_sb[:], Wl[:])
            scoreb_sb = cpool.tile([128, 2 * RATIO], f32)
            nc.sync.dma_start(scoreb_sb[:], scoreb[:])
            attBD_sb = cpool.tile([128, 32 * 64], bf16)
            for b in range(32):
                nc.sync.dma_start(attBD_sb[:, b * 64:(b + 1) * 64],
                                  attBD[:, b * 64:(b + 1) * 64])
            attP_sb = cpool.tile([128, 2 * 64], bf16)
            nc.sync.dma_start(attP_sb[:], attP[:])
            qb_sb = cpool.tile([64, 1], f32)
            nc.sync.dma_start(qb_sb[:], qb[:])
            ipat_sb = cpool.tile([64, RATIO], f32)
            nc.sync.dma_start(ipat_sb[:], ipat[:])
            beff_sb = cpool.tile([C, 1], f32)
            nc.sync.dma_start(beff_sb[:], beff[:])
            ident_sb = cpool.tile([64, 64], bf16)
            nc.sync.dma_start(ident_sb[:], ident[:])
            eselfT_sb = cpool.tile([64, GPC], f32)
            nc.sync.dma_start(eselfT_sb[:], eselfT[:])
            xlselfR_sb = cpool.tile([RATIO, GPC * HEADS * C], bf16)
            nc.sync.dma_start(xlselfR_sb[:], xlselfR[:])

            # ---- Phase A: XlT [(h2,C)=128, n] per head-pair, bf16
            xlT0 = bpool.tile([128, NPC], bf16, tag="xlT0")
            xlT1 = bpool.tile([128, NPC], bf16, tag="xlT1")
            xlT = [xlT0, xlT1]
            for hp in range(2):
                for k in range(NCHUNK):
                    ps = pa_pool.tile([128, PTS], f32, tag="pa")
                    nc.tensor.matmul(
                        ps[:], Wl_sb[:, hp * 128:(hp + 1) * 128],
                        xT_sb[:, k * PTS:(k + 1) * PTS], start=True, stop=True)
                    nc.vector.tensor_copy(xlT[hp][:, k * PTS:(k + 1) * PTS], ps[:])

            # ---- Phase B: Xl [n-tile 128, (h,C)=256] bf16 (aggregation lhsT)
            NT = NPC // 128  # 32 row tiles
            xl_n = bpool.tile([128, NT * HEADS * C], bf16)
            for t in range(NT):
                ps = pa_pool.tile([128, HEADS * C], f32, tag="pa")
                nc.tensor.matmul(
                    ps[:], xT_sb[:, t * 128:(t + 1) * 128], Wl_sb[:],
                    start=True, stop=True)
                nc.vector.tensor_copy(
                    xl_n[:, t * HEADS * C:(t + 1) * HEADS * C], ps[:])

            # ---- Phase C: scores -> e_sb, den
            e_sb = bpool.tile([64, NPC], f32)
            den0 = wpool.tile([64, GPC], f32, tag="den0")
            DVE_C = 0  # walrus: TS hits "too many sync waits"; ACT handles all V blocks
            for g in range(NCHUNK):
                sl = slice(g * PTS, (g + 1) * PTS)
                pA = pa_pool.tile([64, PTS], f32, tag="pA")
                first = True
                for hp in range(2):
                    # P-term: 1.5 * att, all 64 columns
                    nc.tensor.matmul(
                        pA[:], attP_sb[:, hp * 64:(hp + 1) * 64],
                        xlT[hp][:, sl], start=first, stop=False)
                    first = False
                for cc in range(RATIO):
                    for hp in range(2):
                        v = vpool.tile([128, PTS], bf16, tag="v")
                        bias_ap = scoreb_sb[:, hp * RATIO + cc: hp * RATIO + cc + 1]
                        if cc < DVE_C:
                            nc.vector.tensor_scalar(
                                v[:], xlT[hp][:, sl], bias_ap, 0.0,
                                mybir.AluOpType.add, mybir.AluOpType.abs_max)
                        else:
                            nc.scalar.activation(
                                v[:], xlT[hp][:, sl],
                                mybir.ActivationFunctionType.Abs, bias=bias_ap)
                        last = (cc == RATIO - 1) and (hp == 1)
                        nc.tensor.matmul(
                            pA[:], attBD_sb[:, (cc * 2 + hp) * 64:(cc * 2 + hp + 1) * 64],
                            v[:], start=False, stop=last)
                # evacuate with exp: e = exp(0.4*psum + 0.6*Q), den0 = row-sum
                nc.scalar.activation(
                    e_sb[:, sl], pA[:], mybir.ActivationFunctionType.Exp,
                    bias=qb_sb[:], scale=0.4, accum_out=den0[:, g:g + 1])

            # ---- denominators + reciprocals (all graphs at once)
            den = wpool.tile([64, GPC], f32, tag="den")
            nc.vector.tensor_add(den[:], den0[:], eselfT_sb[:])
            rden = wpool.tile([64, GPC], f32, tag="rden")
            nc.vector.reciprocal(rden[:], den[:])
            wself = wpool.tile([64, GPC], f32, tag="wself")
            nc.vector.tensor_mul(wself[:], eselfT_sb[:], rden[:])

            # ---- per graph: normalize, transpose, aggregate
            for g in range(NCHUNK):
                sl = slice(g * PTS, (g + 1) * PTS)
                wn = wpool.tile([64, PTS + RATIO], bf16, tag="wn")
                nc.vector.tensor_scalar(
                    wn[:, 0:PTS], e_sb[:, sl], rden[:, g:g + 1], None,
                    mybir.AluOpType.mult)
                nc.vector.tensor_scalar(
                    wn[:, PTS:], ipat_sb[:], wself[:, g:g + 1], None,
                    mybir.AluOpType.mult)
                # transpose 4x [64,128] + 1x [64,16]
                wT = wpool.tile([128, 5 * 64], bf16, tag="wT")
                for k in range(4):
                    pt = pt_pool.tile([128, 64], bf16, tag="pt")
                    nc.tensor.transpose(
                        pt[:], wn[:, k * 128:(k + 1) * 128], ident_sb[:])
                    nc.vector.tensor_copy(wT[:, k * 64:(k + 1) * 64], pt[:])
                pt = pt_pool.tile([128, 64], bf16, tag="pt")
                nc.tensor.transpose(pt[0:16, :], wn[:, PTS:], ident_sb[:])
                nc.vector.tensor_copy(wT[0:16, 4 * 64:5 * 64], pt[0:16, :])

                pG = pg_pool.tile([64, RATIO], f32, tag="pG")
                first = True
                for h in range(HEADS):
                    hp, j = h // 2, h % 2
                    for k in range(4):
                        t = g * 4 + k
                        blk = wT[:, k * 64 + hp * 32: k * 64 + hp * 32 + 32]
                        rhs = blk.rearrange("p (c two) -> p c two", two=2)[:, :, j]
                        nc.tensor.matmul(
                            pG[:], xl_n[:, t * 256 + h * 64: t * 256 + (h + 1) * 64],
                            rhs, start=first, stop=False)
                        first = False
                    blk = wT[0:16, 4 * 64 + hp * 32: 4 * 64 + hp * 32 + 32]
                    rhs = blk.rearrange("p (c two) -> p c two", two=2)[:, :, j]
                    nc.tensor.matmul(
                        pG[:], xlselfR_sb[:, (g * HEADS + h) * 64:(g * HEADS + h + 1) * 64],
                        rhs, start=False, stop=(h == HEADS - 1))
                ot = wpool.tile([64, RATIO], f32, tag="ot")
                nc.scalar.activation(
                    ot[:], pG[:], mybir.ActivationFunctionType.Identity,
                    bias=beff_sb[:], scale=0.25)
                nc.sync.dma_start(outT[:, g * RATIO:(g + 1) * RATIO], ot[:])

    if not nc.is_finalized():
        nc.finalize()
    return nc


def _leaky(z):
    return np.where(z > 0, z, NEG * z)


def _prep(inputs):
    x = np.asarray(inputs["x"], np.float32)
    xcb = np.asarray(inputs["xcent_base"], np.float32)
    W_l = np.asarray(inputs["W_l"], np.float32)
    b_l = np.asarray(inputs["b_l"], np.float32)
    W_r = np.asarray(inputs["W_r"], np.float32)
    b_r = np.asarray(inputs["b_r"], np.float32)
    att = np.asarray(inputs["att"], np.float32)
    bias = np.asarray(inputs["bias"], np.float32)

    xr = (xcb @ W_r + b_r).reshape(RATIO, HEADS, C)        # [16,4,64]
    blh = b_l.reshape(HEADS, C)
    Q = np.einsum("chk,hk->ch", xr + blh[None], att)        # [16,4]

    # partition map p = hp*32 + 2c + j  ->  (c, h=hp*2+j)
    p_c = np.zeros(64, np.int64)
    p_h = np.zeros(64, np.int64)
    for p in range(64):
        hp, r = divmod(p, 32)
        cc, j = divmod(r, 2)
        p_c[p], p_h[p] = cc, hp * 2 + j

    scoreb = np.zeros((128, 2 * RATIO), np.float32)  # rows (h2,C), col hp*16+c
    for hp in range(2):
        for h2 in range(2):
            h = hp * 2 + h2
            scoreb[h2 * 64:(h2 + 1) * 64, hp * RATIO:(hp + 1) * RATIO] = (xr[:, h, :] + blh[h]).T
    attBD = np.zeros((128, 32 * 64), np.float32)
    for cc in range(RATIO):
        for hp in range(2):
            blk = (cc * 2 + hp) * 64
            for j in range(2):
                h = hp * 2 + j
                m = hp * 32 + 2 * cc + j
                attBD[j * 64:(j + 1) * 64, blk + m] = att[h]
    attP = np.zeros((128, 2 * 64), np.float32)
    for hp in range(2):
        for m in range(64):
            if p_h[m] // 2 == hp:
                j = p_h[m] % 2
                attP[j * 64:(j + 1) * 64, hp * 64 + m] = 1.5 * att[p_h[m]]
    qb = (0.6 * Q[p_c, p_h]).reshape(64, 1).astype(np.float32)
    ipat = (p_c[:, None] == np.arange(RATIO)[None, :]).astype(np.float32)
    beff = (bias + 0.25 * blh.sum(0)).reshape(C, 1).astype(np.float32)
    ident = np.eye(64, dtype=np.float32)

    shared = dict(
        Wl=W_l, scoreb=scoreb, attBD=attBD, attP=attP, qb=qb,
        ipat=ipat, beff=beff, ident=ident)

    per_core = []
    for k in range(NCORES):
        nodes = slice(k * NPC, (k + 1) * NPC)
        xk = x[nodes]
        d0 = k * DPC
        xs = x[d0:d0 + DPC]                       # self-loop source nodes
        xls = xs @ W_l                             # [128, 256] no b_l
        zs = xls.reshape(DPC, HEADS, C) + blh[None] + xr[np.arange(DPC) % RATIO]
        As = np.einsum("dhk,hk->dh", _leaky(zs), att)   # [128, 4]
        es = np.exp(As)
        if k == 0:
            es[:RATIO] = 0.0                       # d<16: self dup of masked edge
        eselfT = np.zeros((64, GPC), np.float32)
        for p in range(64):
            eselfT[p] = es[np.arange(GPC) * RATIO + p_c[p], p_h[p]]
        xlselfR = np.zeros((RATIO, GPC * HEADS * C), np.float32)
        xls4 = xls.reshape(GPC, RATIO, HEADS * C)
        for cc in range(RATIO):
            xlselfR[cc] = xls4[:, cc, :].reshape(-1)
        per_core.append(dict(
            xT=np.ascontiguousarray(xk.T), eselfT=eselfT, xlselfR=xlselfR))
    return shared, per_core, p_c, p_h


def kernel(**inputs):
    from concourse import bass_utils
    import ml_dtypes

    shared, per_core, p_c, p_h = _prep(inputs)

    if "nc" not in _CACHE:
        _CACHE["nc"] = _build_program()
    nc = _CACHE["nc"]

    bf = ml_dtypes.bfloat16
    in_maps = []
    for k in range(NCORES):
        m = {}
        for name, arr in shared.items():
            if name in ("Wl", "attBD", "attP", "ident"):
                m[name] = arr.astype(bf)
            else:
                m[name] = arr.astype(np.float32)
        m["xT"] = per_core[k]["xT"].astype(bf)
        m["eselfT"] = per_core[k]["eselfT"].astype(np.float32)
        m["xlselfR"] = per_core[k]["xlselfR"].astype(bf)
        in_maps.append(m)

    res = bass_utils.run_bass_kernel_spmd(nc, in_maps, list(range(NCORES)))
    outs = []
    for k in range(NCORES):
        oT = np.asarray(res.results[k]["outT"], np.float32)   # [64, 128]
        o = oT.T.reshape(GPC, RATIO, C)
        outs.append(o)
    xcent = np.concatenate(outs, 0).astype(np.float32)        # [64,16,64]
    bdt = np.asarray(inputs["batch"]).dtype
    batchcent = np.repeat(np.arange(NUM_GRAPHS), RATIO).astype(bdt)
    return xcent, batchcent


# revision 13
# speedup vs baseline: 1.0433x; 1.0433x over previous
"""BipartPool GATv2 pooling kernel for trn2, 8-core SPMD, graph-sharded.

Decomposition (per core, 8 graphs = 4096 nodes, 128 centroids):
  A[n,c,h] = sum_C att[h,C] * leaky(z),  z = Xl[n,h,C] + xr[c,h,C] + b_l[h,C]
           = 0.4 * sum_C att*|z|  + 0.6*(P[n,h] + Q[c,h]),   P = sum_C att*Xl
  V_c = |Xl + (xr_c+b_l)|  computed on ACT (Abs) / DVE (tensor_scalar add+abs_max),
  PE reduces V_c with block-diagonal att weights, accumulating all (c,h) rows plus
  the 1.5*P term into one PSUM tile [64,512] per graph; ACT Exp evacuates with
  scale 0.4 and per-partition bias 0.6*Q, accum_out gives the softmax denominator.
  Self-loop edges (PyG add_self_loops crosses graphs) enter as host-computed
  exp(A_self) terms; centroid row d<16 keeps no self term (it duplicates the
  masked dense edge).  Aggregation w^T @ Xl runs per (graph, head) on PE after a
  PE transpose of the normalized weights; head-mean + bias folds into the final
  ACT evacuation (scale 0.25, per-partition bias_eff).
"""

import numpy as np

NUM_GRAPHS = 64
PTS = 512
RATIO = 16
HEADS = 4
C = 64
NEG = 0.2
NCORES = 8
GPC = NUM_GRAPHS // NCORES          # graphs per core = 8
NPC = GPC * PTS                     # nodes per core = 4096
DPC = GPC * RATIO                   # centroids per core = 128

_CACHE = {}


def _build_program():
    import concourse.bass as bass
    import concourse.bacc as bacc
    import concourse.mybir as mybir
    from concourse.tile import TileContext

    f32 = mybir.dt.float32
    bf16 = mybir.dt.bfloat16

    nc = bacc.Bacc(None, target_bir_lowering=True)
    # per-core tensors
    xT = nc.declare_dram_parameter("xT", [C, NPC], bf16, isOutput=False)
    eselfT = nc.declare_dram_parameter("eselfT", [64, GPC], f32, isOutput=False)
    xlselfR = nc.declare_dram_parameter("xlselfR", [RATIO, GPC * HEADS * C], bf16, isOutput=False)
    # shared tensors
    Wl = nc.declare_dram_parameter("Wl", [C, HEADS * C], bf16, isOutput=False)
    scoreb = nc.declare_dram_parameter("scoreb", [128, 2 * RATIO], f32, isOutput=False)
    attBD = nc.declare_dram_parameter("attBD", [128, 32 * 64], bf16, isOutput=False)
    attP = nc.declare_dram_parameter("attP", [128, 2 * 64], bf16, isOutput=False)
    qb = nc.declare_dram_parameter("qb", [64, 1], f32, isOutput=False)
    ipat = nc.declare_dram_parameter("ipat", [64, RATIO], f32, isOutput=False)
    beff = nc.declare_dram_parameter("beff", [C, 1], f32, isOutput=False)
    ident = nc.declare_dram_parameter("ident", [64, 64], bf16, isOutput=False)
    outT = nc.declare_dram_parameter("outT", [C, DPC], f32, isOutput=True)

    NCHUNK = NPC // PTS  # 8 chunks of 512 = one graph each

    with TileContext(nc) as tc:
        with (
            tc.tile_pool(name="const", bufs=1) as cpool,
            tc.tile_pool(name="big", bufs=1) as bpool,
            tc.tile_pool(name="v", bufs=3) as vpool,
            tc.tile_pool(name="w", bufs=2) as wpool,
            tc.tile_pool(name="pa", bufs=2, space="PSUM") as pa_pool,
            tc.tile_pool(name="pt", bufs=2, space="PSUM") as pt_pool,
            tc.tile_pool(name="pg", bufs=2, space="PSUM") as pg_pool,
        ):
            # ---- load constants / inputs
            xT_sb = cpool.tile([C, NPC], bf16)
            for k in range(NPC // PTS):
                nc.sync.dma_start(xT_sb[:, k * PTS:(k + 1) * PTS],
                                  xT[:, k * PTS:(k + 1) * PTS])
            Wl_sb = cpool.tile([C, HEADS * C], bf16)
            nc.sync.dma_start(Wl_sb[:], Wl[:])
            scoreb_sb = cpool.tile([128, 2 * RATIO], f32)
            nc.sync.dma_start(scoreb_sb[:], scoreb[:])
            attBD_sb = cpool.tile([128, 32 * 64], bf16)
            for b in range(32):
                nc.sync.dma_start(attBD_sb[:, b * 64:(b + 1) * 64],
                                  attBD[:, b * 64:(b + 1) * 64])
            attP_sb = cpool.tile([128, 2 * 64], bf16)
            nc.sync.dma_start(attP_sb[:], attP[:])
            qb_sb = cpool.tile([64, 1], f32)
            nc.sync.dma_start(qb_sb[:], qb[:])
            ipat_sb = cpool.tile([64, RATIO], f32)
            nc.sync.dma_start(ipat_sb[:], ipat[:])
            beff_sb = cpool.tile([C, 1], f32)
            nc.sync.dma_start(beff_sb[:], beff[:])
            ident_sb = cpool.tile([64, 64], bf16)
            nc.sync.dma_start(ident_sb[:], ident[:])
            eselfT_sb = cpool.tile([64, GPC], f32)
            nc.sync.dma_start(eselfT_sb[:], eselfT[:])
            xlselfR_sb = cpool.tile([RATIO, GPC * HEADS * C], bf16)
            nc.sync.dma_start(xlselfR_sb[:], xlselfR[:])

            # ---- Phase A: XlT [(h2,C)=128, n] per head-pair, bf16
            xlT0 = bpool.tile([128, NPC], bf16, tag="xlT0")
            xlT1 = bpool.tile([128, NPC], bf16, tag="xlT1")
            xlT = [xlT0, xlT1]
            for hp in range(2):
                for k in range(NCHUNK):
                    ps = pa_pool.tile([128, PTS], f32, tag="pa")
                    nc.tensor.matmul(
                        ps[:], Wl_sb[:, hp * 128:(hp + 1) * 128],
                        xT_sb[:, k * PTS:(k + 1) * PTS], start=True, stop=True)
                    nc.vector.tensor_copy(xlT[hp][:, k * PTS:(k + 1) * PTS], ps[:])

            # ---- Phase B: Xl [n-tile 128, (h,C)=256] bf16 (aggregation lhsT)
            NT = NPC // 128  # 32 row tiles
            xl_n = bpool.tile([128, NT * HEADS * C], bf16)
            for t in range(NT):
                ps = pa_pool.tile([128, HEADS * C], f32, tag="pa")
                nc.tensor.matmul(
                    ps[:], xT_sb[:, t * 128:(t + 1) * 128], Wl_sb[:],
                    start=True, stop=True)
                nc.vector.tensor_copy(
                    xl_n[:, t * HEADS * C:(t + 1) * HEADS * C], ps[:])

            # ---- Phase C: scores -> e_sb, den
            e_sb = bpool.tile([64, NPC], f32)
            den0 = wpool.tile([64, GPC], f32, tag="den0")
            DVE_C = 0  # trn2 HW rejects abs_max as TS Op1; ACT Abs handles all V blocks
            for g in range(NCHUNK):
                sl = slice(g * PTS, (g + 1) * PTS)
                pA = pa_pool.tile([64, PTS], f32, tag="pA")
                first = True
                for hp in range(2):
                    # P-term: 1.5 * att, all 64 columns
                    nc.tensor.matmul(
                        pA[:], attP_sb[:, hp * 64:(hp + 1) * 64],
                        xlT[hp][:, sl], start=first, stop=False)
                    first = False
                for cc in range(RATIO):
                    for hp in range(2):
                        v = vpool.tile([128, PTS], bf16, tag="v")
                        bias_ap = scoreb_sb[:, hp * RATIO + cc: hp * RATIO + cc + 1]
                        if cc < DVE_C:
                            nc.vector.tensor_scalar(
                                v[:], xlT[hp][:, sl], bias_ap, 0.0,
                                mybir.AluOpType.add, mybir.AluOpType.abs_max)
                        else:
                            nc.scalar.activation(
                                v[:], xlT[hp][:, sl],
                                mybir.ActivationFunctionType.Abs, bias=bias_ap)
                        last = (cc == RATIO - 1) and (hp == 1)
                        nc.tensor.matmul(
                            pA[:], attBD_sb[:, (cc * 2 + hp) * 64:(cc * 2 + hp + 1) * 64],
                            v[:], start=False, stop=last)
                # evacuate with exp: e = exp(0.4*psum + 0.6*Q), den0 = row-sum
                nc.scalar.activation(
                    e_sb[:, sl], pA[:], mybir.ActivationFunctionType.Exp,
                    bias=qb_sb[:], scale=0.4, accum_out=den0[:, g:g + 1])

            # ---- denominators + reciprocals (all graphs at once)
            den = wpool.tile([64, GPC], f32, tag="den")
            nc.vector.tensor_add(den[:], den0[:], eselfT_sb[:])
            rden = wpool.tile([64, GPC], f32, tag="rden")
            nc.vector.reciprocal(rden[:], den[:])
            wself = wpool.tile([64, GPC], f32, tag="wself")
            nc.vector.tensor_mul(wself[:], eselfT_sb[:], rden[:])

            # ---- per graph: normalize, transpose, aggregate
            for g in range(NCHUNK):
                sl = slice(g * PTS, (g + 1) * PTS)
                wn = wpool.tile([64, PTS + RATIO], bf16, tag="wn")
                nc.vector.tensor_scalar(
                    wn[:, 0:PTS], e_sb[:, sl], rden[:, g:g + 1], None,
                    mybir.AluOpType.mult)
                nc.vector.tensor_scalar(
                    wn[:, PTS:], ipat_sb[:], wself[:, g:g + 1], None,
                    mybir.AluOpType.mult)
                # transpose 4x [64,128] + 1x [64,16]
                wT = wpool.tile([128, 5 * 64], bf16, tag="wT")
                for k in range(4):
                    pt = pt_pool.tile([128, 64], bf16, tag="pt")
                    nc.tensor.transpose(
                        pt[:], wn[:, k * 128:(k + 1) * 128], ident_sb[:])
                    nc.vector.tensor_copy(wT[:, k * 64:(k + 1) * 64], pt[:])
                pt = pt_pool.tile([128, 64], bf16, tag="pt")
                nc.tensor.transpose(pt[0:16, :], wn[:, PTS:], ident_sb[:])
                nc.vector.tensor_copy(wT[0:16, 4 * 64:5 * 64], pt[0:16, :])

                pG = pg_pool.tile([64, RATIO], f32, tag="pG")
                first = True
                for h in range(HEADS):
                    hp, j = h // 2, h % 2
                    for k in range(4):
                        t = g * 4 + k
                        blk = wT[:, k * 64 + hp * 32: k * 64 + hp * 32 + 32]
                        rhs = blk.rearrange("p (c two) -> p c two", two=2)[:, :, j]
                        nc.tensor.matmul(
                            pG[:], xl_n[:, t * 256 + h * 64: t * 256 + (h + 1) * 64],
                            rhs, start=first, stop=False)
                        first = False
                    blk = wT[0:16, 4 * 64 + hp * 32: 4 * 64 + hp * 32 + 32]
                    rhs = blk.rearrange("p (c two) -> p c two", two=2)[:, :, j]
                    nc.tensor.matmul(
                        pG[:], xlselfR_sb[:, (g * HEADS + h) * 64:(g * HEADS + h + 1) * 64],
                        rhs, start=False, stop=(h == HEADS - 1))
                ot = wpool.tile([64, RATIO], f32, tag="ot")
                nc.scalar.activation(
                    ot[:], pG[:], mybir.ActivationFunctionType.Identity,
                    bias=beff_sb[:], scale=0.25)
                nc.sync.dma_start(outT[:, g * RATIO:(g + 1) * RATIO], ot[:])

    if not nc.is_finalized():
        nc.finalize()
    return nc


def _leaky(z):
    return np.where(z > 0, z, NEG * z)


def _prep(inputs):
    x = np.asarray(inputs["x"], np.float32)
    xcb = np.asarray(inputs["xcent_base"], np.float32)
    W_l = np.asarray(inputs["W_l"], np.float32)
    b_l = np.asarray(inputs["b_l"], np.float32)
    W_r = np.asarray(inputs["W_r"], np.float32)
    b_r = np.asarray(inputs["b_r"], np.float32)
    att = np.asarray(inputs["att"], np.float32)
    bias = np.asarray(inputs["bias"], np.float32)

    xr = (xcb @ W_r + b_r).reshape(RATIO, HEADS, C)        # [16,4,64]
    blh = b_l.reshape(HEADS, C)
    Q = np.einsum("chk,hk->ch", xr + blh[None], att)        # [16,4]

    # partition map p = hp*32 + 2c + j  ->  (c, h=hp*2+j)
    p_c = np.zeros(64, np.int64)
    p_h = np.zeros(64, np.int64)
    for p in range(64):
        hp, r = divmod(p, 32)
        cc, j = divmod(r, 2)
        p_c[p], p_h[p] = cc, hp * 2 + j

    scoreb = np.zeros((128, 2 * RATIO), np.float32)  # rows (h2,C), col hp*16+c
    for hp in range(2):
        for h2 in range(2):
            h = hp * 2 + h2
            scoreb[h2 * 64:(h2 + 1) * 64, hp * RATIO:(hp + 1) * RATIO] = (xr[:, h, :] + blh[h]).T
    attBD = np.zeros((128, 32 * 64), np.float32)
    for cc in range(RATIO):
        for hp in range(2):
            blk = (cc * 2 + hp) * 64
            for j in range(2):
                h = hp * 2 + j
                m = hp * 32 + 2 * cc + j
                attBD[j * 64:(j + 1) * 64, blk + m] = att[h]
    attP = np.zeros((128, 2 * 64), np.float32)
    for hp in range(2):
        for m in range(64):
            if p_h[m] // 2 == hp:
                j = p_h[m] % 2
                attP[j * 64:(j + 1) * 64, hp * 64 + m] = 1.5 * att[p_h[m]]
    qb = (0.6 * Q[p_c, p_h]).reshape(64, 1).astype(np.float32)
    ipat = (p_c[:, None] == np.arange(RATIO)[None, :]).astype(np.float32)
    beff = (bias + 0.25 * blh.sum(0)).reshape(C, 1).astype(np.float32)
    ident = np.eye(64, dtype=np.float32)

    shared = dict(
        Wl=W_l, scoreb=scoreb, attBD=attBD, attP=attP, qb=qb,
        ipat=ipat, beff=beff, ident=ident)

    per_core = []
    for k in range(NCORES):
        nodes = slice(k * NPC, (k + 1) * NPC)
        xk = x[nodes]
        d0 = k * DPC
        xs = x[d0:d0 + DPC]                       # self-loop source nodes
        xls = xs @ W_l                             # [128, 256] no b_l
        zs = xls.reshape(DPC, HEADS, C) + blh[None] + xr[np.arange(DPC) % RATIO]
        As = np.einsum("dhk,hk->dh", _leaky(zs), att)   # [128, 4]
        es = np.exp(As)
        if k == 0:
            es[:RATIO] = 0.0                       # d<16: self dup of masked edge
        eselfT = np.zeros((64, GPC), np.float32)
        for p in range(64):
            eselfT[p] = es[np.arange(GPC) * RATIO + p_c[p], p_h[p]]
        xlselfR = np.zeros((RATIO, GPC * HEADS * C), np.float32)
        xls4 = xls.reshape(GPC, RATIO, HEADS * C)
        for cc in range(RATIO):
            xlselfR[cc] = xls4[:, cc, :].reshape(-1)
        per_core.append(dict(
            xT=np.ascontiguousarray(xk.T), eselfT=eselfT, xlselfR=xlselfR))
    return shared, per_core, p_c, p_h


def kernel(**inputs):
    from concourse import bass_utils
    import ml_dtypes

    shared, per_core, p_c, p_h = _prep(inputs)

    if "nc" not in _CACHE:
        _CACHE["nc"] = _build_program()
    nc = _CACHE["nc"]

    bf = ml_dtypes.bfloat16
    in_maps = []
    for k in range(NCORES):
        m = {}
        for name, arr in shared.items():
            if name in ("Wl", "attBD", "attP", "ident"):
                m[name] = arr.astype(bf)
            else:
                m[name] = arr.astype(np.float32)
        m["xT"] = per_core[k]["xT"].astype(bf)
        m["eselfT"] = per_core[k]["eselfT"].astype(np.float32)
        m["xlselfR"] = per_core[k]["xlselfR"].astype(bf)
        in_maps.append(m)

    res = bass_utils.run_bass_kernel_spmd(nc, in_maps, list(range(NCORES)))
    outs = []
    for k in range(NCORES):
        oT = np.asarray(res.results[k]["outT"], np.float32)   # [64, 128]
        o = oT.T.reshape(GPC, RATIO, C)
        outs.append(o)
    xcent = np.concatenate(outs, 0).astype(np.float32)        # [64,16,64]
    bdt = np.asarray(inputs["batch"]).dtype
    batchcent = np.repeat(np.arange(NUM_GRAPHS), RATIO).astype(bdt)
    return xcent, batchcent
